# revision 37
# baseline (speedup 1.0000x reference)
"""Trainium2 Bass kernel for SAM-style decomposed rel-pos attention (v6).

Problem: B=1, HW=2304 (48x48), NH=16 heads, DH=64, D=1024, f32 in/out.
  attn = softmax(q*scale @ k^T + rel_h[qh,kh] + rel_w[qw,kw]); out = attn @ v

Strategy (8 NeuronCores, SPMD): 2 heads per core. Key ideas:
- rel_h folded into the score matmul (one-hot Eh rows + K^T stationary;
  gathered rel_h rows + Q^T moving) -> 18 score matmuls per (head, chunk).
- rel_w applied multiplicatively after exp: P = exp(S_qk+relh) * Ew with
  only 3 row-rotations of exp_relw -> [128, 3, HW] patterns tile.
- Diagonal gathers via scratch-DRAM roundtrip with a -2256-stride AP.
- exp on ScalarE in [128, 3, qn] groups; PV matmuls lag PV_LAG groups.

v6 changes:
- TRANSPOSED PV: P tiles are the stationary operand ([128k, 128q] slices),
  V the moving ([128k, 65]); output lands [q-partition, dh] in one PSUM
  bank per chunk, accumulated over all 18 k-tiles. The ones-column
  denominator arrives per-partition, so softmax division is reciprocal +
  a per-partition tensor_scalar multiply: the old den-transpose DMAs,
  reciprocal-broadcast matmul, and deferred-epilogue machinery all go.
  PE cost is neutral: 324 LDWEIGHTS-paced slots x 128 cycles == the old
  column-paced PV stream.
- Schraudolph exp on DVE for every 5th group: P = bitcast_bf16(int16(
  S*(128/ln2) + (127*128 - 7))), ~1.8% rms on those tiles (validated
  0.8% end-to-end), relieving the ScalarE exp floor (the true wall).
- Startup pipelining: rqs/rqw/lhsT split across both DMA rings up front;
  vt + head-1 loads stream in mid-loop; T1h pieces 0-1 gate the first
  score matmul; PV lag 6 on chunk 0 gives the T2w->exp->pats chain
  score-matmul runway. ACT exp table preloaded via a dummy activation.
"""

import sys

sys.path.insert(0, "/opt/trn_rl_repo")

import numpy as np
import ml_dtypes

from concourse import bacc, mybir, tile
from concourse.tile import add_dep_helper
from concourse.bass_utils import run_bass_kernel_spmd

BF16 = mybir.dt.bfloat16
F32 = mybir.dt.float32
I16 = mybir.dt.int16
BF = ml_dtypes.bfloat16

H = 48
W = 48
HW = H * W          # 2304
DH = 64
NH = 16
N_CORES = 8
HPC = 2             # heads per core
KT = HW // 128      # 18 k tiles
QCHUNKS = [(0, 512), (512, 512), (1024, 512), (1536, 512), (2048, 256)]
NG = KT // 3        # 6 groups of 3 k-tiles
PV_LAG = 4          # PV runs this many groups behind the score matmuls

# Schraudolph bf16 exp: bitcast(int16(x*128/ln2 + 127*128 - C)), C=7
SCH_C1 = 128.0 / float(np.log(2.0))
SCH_C2 = 127.0 * 128.0 - 7.0
SCH_EVERY = 6       # 0 = disabled; N = every Nth group on DVE

# gather piece ranges: piece c covers blocks [lo, hi) whose table cols
# fit inside table-store chunks 0..c (QCHUNKS cols)
PIECES = [(0, 10), (10, 21), (21, 32), (32, 42), (42, 48)]

_NC = None


def _build_nc():
    nc = bacc.Bacc(None, target_bir_lowering=False)

    lhs_p = nc.dram_tensor("lhs_p", [128, HPC * HW], BF16, kind="ExternalInput")
    rqs_p = nc.dram_tensor("rqs_p", [80, HPC * HW], BF16, kind="ExternalInput")
    rqw_p = nc.dram_tensor("rqw_p", [64, HPC * HW], BF16, kind="ExternalInput")
    v_til = nc.dram_tensor("v_til", [128, HPC * KT * 65], BF16, kind="ExternalInput")
    rhv = nc.dram_tensor("rhv", [64, 95], BF16, kind="ExternalInput")
    rwv = nc.dram_tensor("rwv", [64, 95], BF16, kind="ExternalInput")
    out_t = nc.dram_tensor("out_t", [HPC * HW, 64], F32, kind="ExternalOutput")
    t1d = [nc.dram_tensor(f"t1d{h}", [95, HW], BF16, kind="Internal") for h in range(HPC)]
    t2d = [nc.dram_tensor(f"t2d{h}", [95, HW], BF16, kind="Internal") for h in range(HPC)]

    Exp = mybir.ActivationFunctionType.Exp
    Copy = mybir.ActivationFunctionType.Copy
    MULT = mybir.AluOpType.mult
    ADD = mybir.AluOpType.add

    with tile.TileContext(nc) as tc:
        with (
            tc.tile_pool(name="const", bufs=1) as cpool,
            tc.tile_pool(name="stack", bufs=2) as spool,
            tc.tile_pool(name="p1t", bufs=4) as p1pool,
            tc.tile_pool(name="p1i", bufs=2) as p1ipool,
            tc.tile_pool(name="p2t", bufs=10) as p2pool,
            tc.tile_pool(name="epil", bufs=3) as epool,
            tc.tile_pool(name="ps_s", bufs=2, space="PSUM") as ps_s,
            tc.tile_pool(name="ps_o", bufs=2, space="PSUM") as ps_o,
        ):
            # rel tables on both partition halves: rows 0:64 feed the T2w
            # matmuls (rqw lives on partitions 0:64), rows 64:128 feed T1h
            # (q-major Q^T lives on rq partitions 64:128).
            rhv_sb = cpool.tile([128, 95], BF16, tag="rhv")
            rwv_sb = cpool.tile([128, 95], BF16, tag="rwv")
            ones1 = cpool.tile([1, 64], BF16, tag="ones1")
            nc.sync.dma_start(rwv_sb[0:64, :], rwv[:, :])
            nc.gpsimd.dma_start(rhv_sb[64:128, :], rhv[:, :])
            nc.gpsimd.memset(ones1[:], 1.0)

            hs = [dict() for _ in range(HPC)]

            def ph1_tiles(hh):
                s = hs[hh]
                s["lhsT"] = spool.tile([128, HW], BF16, tag="lhsT", name=f"lhsT{hh}")
                s["rq"] = spool.tile([128, HW], BF16, tag="rq", name=f"rq{hh}")
                s["rqw"] = spool.tile([64, HW], BF16, tag="rqw", name=f"rqw{hh}")
                s["vt"] = spool.tile([128, KT * 65], BF16, tag="vt", name=f"vt{hh}")
                s["t1sb"] = spool.tile([95, HW], BF16, tag="t1sb", name=f"t1sb{hh}")
                s["t2sb"] = spool.tile([95, HW], BF16, tag="t2sb", name=f"t2sb{hh}")
                s["relw"] = spool.tile([48, HW], BF16, tag="relw", name=f"relw{hh}")
                s["expw"] = spool.tile([48, HW], BF16, tag="expw", name=f"expw{hh}")
                s["pats"] = spool.tile([128, 3, HW], BF16, tag="pats", name=f"pats{hh}")

            def ph1_load_q(hh):
                # rq (q-major Q^T + zero rows) and rqw (w-major): the prep
                # critical path. Halves split across both rings.
                s = hs[hh]
                o = hh * HW
                nc.sync.dma_start(s["rq"][48:128, 0:1152], rqs_p[:, o : o + 1152])
                nc.gpsimd.dma_start(
                    s["rq"][48:128, 1152:HW], rqs_p[:, o + 1152 : o + HW]
                )
                nc.sync.dma_start(s["rqw"][:, 0:1152], rqw_p[:, o : o + 1152])
                nc.gpsimd.dma_start(s["rqw"][:, 1152:HW], rqw_p[:, o + 1152 : o + HW])

            def ph1_load_q_min(hh):
                # startup-minimal: only the halves the first tab matmuls and
                # first two chunks touch; the rqs second half comes at slot 0
                s = hs[hh]
                o = hh * HW
                nc.sync.dma_start(s["rq"][48:128, 0:1152], rqs_p[:, o : o + 1152])
                nc.sync.dma_start(s["rqw"][:, 0:1152], rqw_p[:, o : o + 1152])
                nc.gpsimd.dma_start(s["rqw"][:, 1152:HW], rqw_p[:, o + 1152 : o + HW])

            def ph1_load_q_rest(hh):
                s = hs[hh]
                o = hh * HW
                nc.gpsimd.dma_start(
                    s["rq"][48:128, 1152:HW], rqs_p[:, o + 1152 : o + HW]
                )

            def ph1_load_lhs(hh, c0=0, c1=HW):
                s = hs[hh]
                o = hh * HW
                if c1 - c0 > 1152:
                    nc.sync.dma_start(
                        s["lhsT"][:, c0:1152], lhs_p[:, o + c0 : o + 1152]
                    )
                    nc.gpsimd.dma_start(
                        s["lhsT"][:, 1152:c1], lhs_p[:, o + 1152 : o + c1]
                    )
                else:
                    nc.sync.dma_start(s["lhsT"][:, c0:c1], lhs_p[:, o + c0 : o + c1])

            def ph1_load_vt(hh):
                s = hs[hh]
                eng = nc.gpsimd if hh == 0 else nc.sync
                eng.dma_start(s["vt"][:, :], v_til[:, hh * KT * 65 : (hh + 1) * KT * 65])

            def tabw(hh, ci, cast_act=False):
                # T2w chunk: matmul -> cast -> DRAM store -> diagonal gather.
                # Store and gather share a ring: queue FIFO ordering makes
                # the store->gather dependency nearly latency-free.
                s = hs[hh]
                ringA = ringB = nc.gpsimd if hh == 0 else nc.sync
                (q0, qn) = QCHUNKS[ci]
                tp = ps_s.tile([128, 3, 512], F32, tag="s", name=f"tpw{hh}_{ci}")
                nc.tensor.matmul(
                    tp[0:95, 0, 0:qn], rwv_sb[0:64, :],
                    s["rqw"][:, q0 : q0 + qn], start=True, stop=True,
                )
                if cast_act:
                    nc.scalar.activation(
                        s["t2sb"][:, q0 : q0 + qn], tp[0:95, 0, 0:qn], Copy
                    )
                else:
                    nc.vector.tensor_copy(s["t2sb"][:, q0 : q0 + qn], tp[0:95, 0, 0:qn])
                ringA.dma_start(
                    t2d[hh][0:95, q0 : q0 + qn], s["t2sb"][0:95, q0 : q0 + qn]
                )
                (ba, bb) = PIECES[ci]
                nbl = bb - ba
                # rel_w (w-major): relw[j, (w,h)] = T2w[47-w+j, 48w+h]
                dstw = s["relw"][0:48, 48 * ba : 48 * bb].rearrange(
                    "p (w h) -> p w h", h=48
                )
                srcw = t2d[hh][47 - ba : 95 - ba, 48 * ba : 48 * bb].rearrange(
                    "j (w h) -> j w h", h=48
                )
                srcw.ap[1] = [-2256, nbl]
                ringB.dma_start(dstw, srcw)

            def tabh(hh, ci, cast_act=False):
                # T1h chunk: matmul -> cast -> DRAM store -> diagonal gather
                s = hs[hh]
                ringA = ringB = nc.sync if hh == 0 else nc.gpsimd
                (q0, qn) = QCHUNKS[ci]
                tp = ps_s.tile([128, 3, 512], F32, tag="s", name=f"tph{hh}_{ci}")
                nc.tensor.matmul(
                    tp[0:95, 0, 0:qn], rhv_sb[64:128, :],
                    s["rq"][64:128, q0 : q0 + qn], start=True, stop=True,
                )
                if cast_act:
                    nc.scalar.activation(
                        s["t1sb"][:, q0 : q0 + qn], tp[0:95, 0, 0:qn], Copy
                    )
                else:
                    nc.vector.tensor_copy(s["t1sb"][:, q0 : q0 + qn], tp[0:95, 0, 0:qn])
                ringA.dma_start(
                    t1d[hh][0:95, q0 : q0 + qn], s["t1sb"][0:95, q0 : q0 + qn]
                )
                (ba, bb) = PIECES[ci]
                nbl = bb - ba
                # rel_h: rq[j, (h,w)] = T1h[47-h+j, 48h+w], h in [ba, bb)
                dsth = s["rq"][0:48, 48 * ba : 48 * bb].rearrange(
                    "p (h w) -> p h w", w=48
                )
                srch = t1d[hh][47 - ba : 95 - ba, 48 * ba : 48 * bb].rearrange(
                    "j (h w) -> j h w", w=48
                )
                srch.ap[1] = [-2256, nbl]
                ringB.dma_start(dsth, srch)

            def relw_exp(hh, half):
                # exp with an un-permuting (w-major -> q-major) input AP
                s = hs[hh]
                h0 = half * 24  # h-blocks 0:24/24:48 -> cols 0:1152/1152:2304
                co, cn = h0 * 48, 1152
                nc.scalar.activation(
                    s["expw"][:, co : co + cn].rearrange("p (h w) -> p h w", w=48),
                    s["relw"][:, :].rearrange("p (w h) -> p h w", w=48)[:, h0 : h0 + 24, :],
                    Exp,
                )

            def pats_copy(hh, co, cn):
                # 3 row-rotations (k-tile offsets 0/32/16) of exp_relw
                s = hs[hh]
                ndma = 0
                for j, off in enumerate((0, 32, 16)):
                    p = 0
                    while p < 128:
                        r0 = (p + off) % 48
                        n = min(48 - r0, 128 - p)
                        eng = (nc.sync, nc.gpsimd)[(hh + ndma) % 2]
                        eng.dma_start(
                            s["pats"][p : p + n, j, co : co + cn],
                            s["expw"][r0 : r0 + n, co : co + cn],
                        )
                        p += n
                        ndma += 1

            # ---- main loop machinery (shared across heads) ----
            # PV pipeline: each chunk's PVs run during the NEXT chunk, one
            # full q-block (all 18 k-tiles, sequential PSUM accumulation)
            # per score group. Interleaving accumulation regions within a
            # PSUM bank corrupts all but the last region, so each block's
            # 18-matmul accumulation must be contiguous in the PE stream.
            st = {"pend": []}

            def epilogue(e):
                hh, ci, q0, qn, o_ps = e["hh"], e["ci"], e["q0"], e["qn"], e["o_ps"]
                nb = qn // 128
                rect = epool.tile([128, 4], F32, tag="rect", name=f"rect{hh}_{ci}")
                ot = epool.tile([128, 256], F32, tag="ot", name=f"ot{hh}_{ci}")
                for b in range(nb):
                    nc.vector.reciprocal(
                        rect[:, b : b + 1],
                        o_ps[:, 128 * b + 64 : 128 * b + 65],
                    )
                    nc.vector.tensor_scalar_mul(
                        ot[:, 64 * b : 64 * (b + 1)],
                        o_ps[:, 128 * b : 128 * b + 64],
                        rect[:, b : b + 1],
                    )
                dmae = nc.sync if ci % 2 == 0 else nc.gpsimd
                r0 = hh * HW + q0
                dmae.dma_start(
                    out_t[r0 : r0 + qn, :].rearrange("(b p) d -> p b d", p=128),
                    ot[:, 0 : 64 * nb].rearrange("p (b d) -> p b d", d=64),
                )

            def pv_step(last_mm):
                if not st["pend"]:
                    return
                e = st["pend"][0]
                b, o_ps, p2s, vt = e["b"], e["o_ps"], e["p2s"], e["vt"]
                for kt in range(KT):
                    g, j = divmod(kt, 3)
                    pv = nc.tensor.matmul(
                        o_ps[:, 128 * b : 128 * b + 65],
                        p2s[g][:, j, 128 * b : 128 * b + 128],
                        vt[:, kt * 65 : (kt + 1) * 65],
                        start=(kt == 0), stop=(kt == KT - 1),
                    )
                    if last_mm is not None:
                        add_dep_helper(pv.ins, last_mm.ins, sync=False,
                                       reason="pv after score mms")
                e["b"] += 1
                if e["b"] == e["qn"] // 128:
                    epilogue(e)
                    st["pend"].pop(0)

            def chunk(hh, ci, inserts=None):
                s = hs[hh]
                (q0, qn) = QCHUNKS[ci]
                o_ps = ps_o.tile([128, 512], F32, tag="o", name=f"o{hh}_{ci}")
                p2s = []
                for g in range(NG):
                    if inserts:
                        inserts.pop(0)()
                    s_ps = ps_s.tile([128, 3, 512], F32, tag="s",
                                     name=f"s{hh}_{ci}_{g}")
                    last_mm = None
                    for j in range(3):
                        kt = 3 * g + j
                        last_mm = nc.tensor.matmul(
                            s_ps[:, j, 0:qn],
                            s["lhsT"][:, kt * 128 : (kt + 1) * 128],
                            s["rq"][:, q0 : q0 + qn],
                            start=True, stop=True,
                        )
                    pv_step(last_mm)
                    if SCH_EVERY and (ci * NG + g) % SCH_EVERY == 2:
                        # Schraudolph exp on DVE (every 5th group)
                        p1i = p1ipool.tile([128, 3, 512], I16, tag="p1i")
                        nc.vector.tensor_scalar(
                            p1i[:, :, 0:qn], s_ps[:, :, 0:qn],
                            SCH_C1, SCH_C2, MULT, ADD,
                        )
                        p1v = p1i.bitcast(BF16)
                    else:
                        p1 = p1pool.tile([128, 3, 512], BF16, tag="p1")
                        nc.scalar.activation(p1[:, :, 0:qn], s_ps[:, :, 0:qn], Exp)
                        p1v = p1
                    p2 = p2pool.tile([128, 3, 512], BF16, tag="p2")
                    nc.vector.tensor_mul(
                        p2[:, :, 0:qn], p1v[:, :, 0:qn],
                        s["pats"][:, :, q0 : q0 + qn],
                    )
                    p2s.append(p2)
                st["pend"].append(dict(hh=hh, ci=ci, q0=q0, qn=qn, o_ps=o_ps,
                                       p2s=p2s, vt=s["vt"], b=0))

            def finish_all():
                while st["pend"]:
                    pv_step(None)

            # ---- schedule ----
            ph1_tiles(0)
            ph1_tiles(1)
            # preload the ACT exp table while DMAs are in flight
            warm = epool.tile([1, 64], BF16, tag="warm")
            nc.scalar.activation(warm[0:1, 0:64], ones1[0:1, 0:64], Exp)
            # startup-critical loads only: first halves of Q layouts + the
            # first 3 k-tiles of the K stack; the rest streams in at slots
            # 0-1 so the rel-table stores aren't queued behind them.
            ph1_load_q_min(0)
            ph1_load_lhs(0, 0, 384)
            # prep chains: T1h pieces 0-1 gate the first score matmul (casts
            # on the idle ACT; Copy shares the Exp table set); the T2w chain
            # (casts on the idle DVE) gates the first p2 multiply, with a
            # full chunk of score-matmul runway before that.
            tabh(0, 0, cast_act=True)
            tabh(0, 1, cast_act=True)
            for c in range(5):
                tabw(0, c)
            relw_exp(0, 0)
            pats_copy(0, 0, 1152)

            nop = lambda: None
            ins0 = [
                # during head-0 chunk 0 (6 slots)
                lambda: (ph1_load_q_rest(0), ph1_load_lhs(0, 384, HW),
                         ph1_load_vt(0)),
                lambda: ph1_load_q(1),
                lambda: tabh(0, 2),
                lambda: (ph1_load_lhs(1), ph1_load_vt(1)),
                nop,
                nop,
                # chunk 1
                lambda: tabw(1, 0),
                lambda: relw_exp(0, 1),
                lambda: tabw(1, 1),
                lambda: pats_copy(0, 1152, 1152),
                lambda: tabw(1, 2),
                lambda: tabh(0, 3),
                # chunk 2
                lambda: tabw(1, 3),
                nop,
                lambda: tabw(1, 4),
                lambda: tabh(0, 4),
                lambda: relw_exp(1, 0),
                nop,
                # chunk 3
                lambda: pats_copy(1, 0, 1152),
                nop,
                lambda: tabh(1, 0),
                lambda: relw_exp(1, 1),
                nop,
                lambda: tabh(1, 1),
                # chunk 4
                lambda: pats_copy(1, 1152, 1152),
                nop,
                lambda: tabh(1, 2),
                nop,
                lambda: tabh(1, 3),
                nop,
            ]
            ins1 = [
                lambda: tabh(1, 4),
            ] + [nop] * 29

            for ci in range(5):
                chunk(0, ci, ins0)
            for ci in range(5):
                chunk(1, ci, ins1)
            finish_all()

    nc.compile()
    return nc


def _get_nc():
    global _NC
    if _NC is None:
        _NC = _build_nc()
    return _NC


def _host_prep(q, k, v, rel_pos_h, rel_pos_w):
    q2 = np.asarray(q, np.float32).reshape(HW, NH * DH)
    k2 = np.asarray(k, np.float32).reshape(HW, NH * DH)
    v2 = np.asarray(v, np.float32).reshape(HW, NH * DH)
    rph = np.asarray(rel_pos_h, np.float32)
    rpw = np.asarray(rel_pos_w, np.float32)

    ar = np.arange(48)
    # reversed rel tables, x8 cancels the 0.125 q scale
    rhv = np.ascontiguousarray((8.0 * rph[::-1]).T).astype(BF)   # (64, 95)
    rwv = np.ascontiguousarray((8.0 * rpw[::-1]).T).astype(BF)
    kk = np.arange(HW)
    eh = np.zeros((64, HW), np.float32)
    eh[:48] = kk[None, :] // 48 == ar[:, None]
    eh = eh.astype(BF)

    onecol = np.ones((HW, 1), np.float32)
    in_maps = []
    for c in range(N_CORES):
        sl = slice(c * 128, (c + 1) * 128)
        qs = (q2[:, sl].T * 0.125).astype(BF)                    # (128, HW)
        qw = np.ascontiguousarray(
            qs.reshape(128, 48, 48).transpose(0, 2, 1)
        ).reshape(128, HW)                                       # w-major cols
        ks = k2[:, sl].T.astype(BF)
        lhs_p = np.zeros((128, HPC, HW), BF)
        rqs_p = np.zeros((80, HPC, HW), BF)
        rqw_p = np.zeros((64, HPC, HW), BF)
        vparts = []
        for hh in range(HPC):
            r0, r1 = hh * 64, (hh + 1) * 64
            lhs_p[0:64, hh, :] = eh
            lhs_p[64:128, hh, :] = ks[r0:r1]
            rqs_p[16:80, hh, :] = qs[r0:r1]
            rqw_p[:, hh, :] = qw[r0:r1]
            vh = v2[:, c * 128 + hh * 64 : c * 128 + (hh + 1) * 64]
            va = np.concatenate([vh, onecol], axis=1)            # (HW, 65)
            vparts.append(va.reshape(KT, 128, 65).transpose(1, 0, 2).reshape(128, KT * 65))
        v_til = np.concatenate(vparts, axis=1).astype(BF)        # (128, 2*18*65)
        in_maps.append(
            dict(
                lhs_p=lhs_p.reshape(128, HPC * HW),
                rqs_p=rqs_p.reshape(80, HPC * HW),
                rqw_p=rqw_p.reshape(64, HPC * HW),
                v_til=v_til, rhv=rhv, rwv=rwv,
            )
        )
    return in_maps


def _assemble(results):
    # out_t per core: [HPC*HW, 64] f32, head hh in rows hh*HW:(hh+1)*HW
    full = np.empty((HW, NH * DH), np.float32)
    for c, r in enumerate(results):
        o = np.asarray(r["out_t"], np.float32)
        for hh in range(HPC):
            full[:, c * 128 + hh * 64 : c * 128 + (hh + 1) * 64] = \
                o[hh * HW : (hh + 1) * HW, :]
    return full.reshape(1, H, W, NH * DH)


def kernel(q, k, v, rel_pos_h, rel_pos_w):
    nc = _get_nc()
    in_maps = _host_prep(q, k, v, rel_pos_h, rel_pos_w)
    res = run_bass_kernel_spmd(nc, in_maps, core_ids=list(range(N_CORES)))
    return _assemble(res.results)


# revision 40
# speedup vs baseline: 1.0073x; 1.0073x over previous
"""Trainium2 Bass kernel for SAM-style decomposed rel-pos attention (v6).

Problem: B=1, HW=2304 (48x48), NH=16 heads, DH=64, D=1024, f32 in/out.
  attn = softmax(q*scale @ k^T + rel_h[qh,kh] + rel_w[qw,kw]); out = attn @ v

Strategy (8 NeuronCores, SPMD): 2 heads per core. Key ideas:
- rel_h folded into the score matmul (one-hot Eh rows + K^T stationary;
  gathered rel_h rows + Q^T moving) -> 18 score matmuls per (head, chunk).
- rel_w applied multiplicatively after exp: P = exp(S_qk+relh) * Ew with
  only 3 row-rotations of exp_relw -> [128, 3, HW] patterns tile.
- Diagonal gathers via scratch-DRAM roundtrip with a -2256-stride AP.
- exp on ScalarE in [128, 3, qn] groups; PV matmuls lag PV_LAG groups.

v6 changes:
- TRANSPOSED PV: P tiles are the stationary operand ([128k, 128q] slices),
  V the moving ([128k, 65]); output lands [q-partition, dh] in one PSUM
  bank per chunk, accumulated over all 18 k-tiles. The ones-column
  denominator arrives per-partition, so softmax division is reciprocal +
  a per-partition tensor_scalar multiply: the old den-transpose DMAs,
  reciprocal-broadcast matmul, and deferred-epilogue machinery all go.
  PE cost is neutral: 324 LDWEIGHTS-paced slots x 128 cycles == the old
  column-paced PV stream.
- Schraudolph exp on DVE for every 5th group: P = bitcast_bf16(int16(
  S*(128/ln2) + (127*128 - 7))), ~1.8% rms on those tiles (validated
  0.8% end-to-end), relieving the ScalarE exp floor (the true wall).
- Startup pipelining: rqs/rqw/lhsT split across both DMA rings up front;
  vt + head-1 loads stream in mid-loop; T1h pieces 0-1 gate the first
  score matmul; PV lag 6 on chunk 0 gives the T2w->exp->pats chain
  score-matmul runway. ACT exp table preloaded via a dummy activation.
"""

import sys

sys.path.insert(0, "/opt/trn_rl_repo")

import numpy as np
import ml_dtypes

from concourse import bacc, mybir, tile
from concourse.tile import add_dep_helper
from concourse.bass_utils import run_bass_kernel_spmd

BF16 = mybir.dt.bfloat16
F32 = mybir.dt.float32
I16 = mybir.dt.int16
BF = ml_dtypes.bfloat16

H = 48
W = 48
HW = H * W          # 2304
DH = 64
NH = 16
N_CORES = 8
HPC = 2             # heads per core
KT = HW // 128      # 18 k tiles
QCHUNKS = [(0, 512), (512, 512), (1024, 512), (1536, 512), (2048, 256)]
NG = KT // 3        # 6 groups of 3 k-tiles
PV_LAG = 4          # PV runs this many groups behind the score matmuls

# Schraudolph bf16 exp: bitcast(int16(x*128/ln2 + 127*128 - C)), C=7
SCH_C1 = 128.0 / float(np.log(2.0))
SCH_C2 = 127.0 * 128.0 - 7.0
SCH_EVERY = 6       # 0 = disabled; N = every Nth group on DVE

# gather piece ranges: piece c covers blocks [lo, hi) whose table cols
# fit inside table-store chunks 0..c (QCHUNKS cols)
PIECES = [(0, 10), (10, 21), (21, 32), (32, 42), (42, 48)]

_NC = None


def _build_nc():
    nc = bacc.Bacc(None, target_bir_lowering=False)

    lhs_p = nc.dram_tensor("lhs_p", [128, HPC * HW], BF16, kind="ExternalInput")
    rqs_p = nc.dram_tensor("rqs_p", [80, HPC * HW], BF16, kind="ExternalInput")
    rqw_p = nc.dram_tensor("rqw_p", [64, HPC * HW], BF16, kind="ExternalInput")
    v_til = nc.dram_tensor("v_til", [128, HPC * KT * 65], BF16, kind="ExternalInput")
    rhv = nc.dram_tensor("rhv", [64, 95], BF16, kind="ExternalInput")
    rwv = nc.dram_tensor("rwv", [64, 95], BF16, kind="ExternalInput")
    out_t = nc.dram_tensor("out_t", [HPC * HW, 64], F32, kind="ExternalOutput")
    t1d = [nc.dram_tensor(f"t1d{h}", [95, HW], BF16, kind="Internal") for h in range(HPC)]
    t2d = [nc.dram_tensor(f"t2d{h}", [95, HW], BF16, kind="Internal") for h in range(HPC)]

    Exp = mybir.ActivationFunctionType.Exp
    Copy = mybir.ActivationFunctionType.Copy
    MULT = mybir.AluOpType.mult
    ADD = mybir.AluOpType.add

    with tile.TileContext(nc) as tc:
        with (
            tc.tile_pool(name="const", bufs=1) as cpool,
            tc.tile_pool(name="stack", bufs=2) as spool,
            tc.tile_pool(name="p1t", bufs=4) as p1pool,
            tc.tile_pool(name="p1i", bufs=2) as p1ipool,
            tc.tile_pool(name="p2t", bufs=10) as p2pool,
            tc.tile_pool(name="epil", bufs=3) as epool,
            tc.tile_pool(name="ps_s", bufs=2, space="PSUM") as ps_s,
            tc.tile_pool(name="ps_o", bufs=2, space="PSUM") as ps_o,
        ):
            # rel tables on both partition halves: rows 0:64 feed the T2w
            # matmuls (rqw lives on partitions 0:64), rows 64:128 feed T1h
            # (q-major Q^T lives on rq partitions 64:128).
            rhv_sb = cpool.tile([128, 95], BF16, tag="rhv")
            rwv_sb = cpool.tile([128, 95], BF16, tag="rwv")
            ones1 = cpool.tile([1, 64], BF16, tag="ones1")
            nc.sync.dma_start(rwv_sb[0:64, :], rwv[:, :])
            nc.gpsimd.dma_start(rhv_sb[64:128, :], rhv[:, :])
            nc.gpsimd.memset(ones1[:], 1.0)

            hs = [dict() for _ in range(HPC)]

            def ph1_tiles(hh):
                s = hs[hh]
                s["lhsT"] = spool.tile([128, HW], BF16, tag="lhsT", name=f"lhsT{hh}")
                s["rq"] = spool.tile([128, HW], BF16, tag="rq", name=f"rq{hh}")
                s["rqw"] = spool.tile([64, HW], BF16, tag="rqw", name=f"rqw{hh}")
                s["vt"] = spool.tile([128, KT * 65], BF16, tag="vt", name=f"vt{hh}")
                s["t1sb"] = spool.tile([95, HW], BF16, tag="t1sb", name=f"t1sb{hh}")
                s["t2sb"] = spool.tile([95, HW], BF16, tag="t2sb", name=f"t2sb{hh}")
                s["relw"] = spool.tile([48, HW], BF16, tag="relw", name=f"relw{hh}")
                s["expw"] = spool.tile([48, HW], BF16, tag="expw", name=f"expw{hh}")
                s["pats"] = spool.tile([128, 3, HW], BF16, tag="pats", name=f"pats{hh}")

            def ph1_load_q(hh):
                # rq (q-major Q^T + zero rows) and rqw (w-major): the prep
                # critical path. Halves split across both rings.
                s = hs[hh]
                o = hh * HW
                nc.sync.dma_start(s["rq"][48:128, 0:1152], rqs_p[:, o : o + 1152])
                nc.gpsimd.dma_start(
                    s["rq"][48:128, 1152:HW], rqs_p[:, o + 1152 : o + HW]
                )
                nc.sync.dma_start(s["rqw"][:, 0:1152], rqw_p[:, o : o + 1152])
                nc.gpsimd.dma_start(s["rqw"][:, 1152:HW], rqw_p[:, o + 1152 : o + HW])

            def ph1_load_q_min(hh):
                # startup-minimal: only the halves the first tab matmuls and
                # first two chunks touch; the rqs second half comes at slot 0
                s = hs[hh]
                o = hh * HW
                nc.sync.dma_start(s["rq"][48:128, 0:1152], rqs_p[:, o : o + 1152])
                nc.sync.dma_start(s["rqw"][:, 0:1152], rqw_p[:, o : o + 1152])
                nc.gpsimd.dma_start(s["rqw"][:, 1152:HW], rqw_p[:, o + 1152 : o + HW])

            def ph1_load_q_rest(hh):
                s = hs[hh]
                o = hh * HW
                nc.gpsimd.dma_start(
                    s["rq"][48:128, 1152:HW], rqs_p[:, o + 1152 : o + HW]
                )

            def ph1_load_lhs(hh, c0=0, c1=HW):
                s = hs[hh]
                o = hh * HW
                if c1 - c0 > 1152:
                    nc.sync.dma_start(
                        s["lhsT"][:, c0:1152], lhs_p[:, o + c0 : o + 1152]
                    )
                    nc.gpsimd.dma_start(
                        s["lhsT"][:, 1152:c1], lhs_p[:, o + 1152 : o + c1]
                    )
                else:
                    nc.sync.dma_start(s["lhsT"][:, c0:c1], lhs_p[:, o + c0 : o + c1])

            def ph1_load_vt(hh):
                s = hs[hh]
                eng = nc.gpsimd if hh == 0 else nc.sync
                eng.dma_start(s["vt"][:, :], v_til[:, hh * KT * 65 : (hh + 1) * KT * 65])

            def tabw(hh, ci):
                # T2w chunk: matmul -> EXP-cast -> DRAM store -> diagonal
                # gather. The table is exponentiated at the PSUM cast (exp
                # commutes with the gather/permute), so the un-permuting
                # pass later is a plain copy instead of a strided exp.
                # Store and gather share a ring: queue FIFO ordering makes
                # the store->gather dependency nearly latency-free.
                s = hs[hh]
                ringA = ringB = nc.gpsimd if hh == 0 else nc.sync
                (q0, qn) = QCHUNKS[ci]
                tp = ps_s.tile([128, 3, 512], F32, tag="s", name=f"tpw{hh}_{ci}")
                nc.tensor.matmul(
                    tp[0:95, 0, 0:qn], rwv_sb[0:64, :],
                    s["rqw"][:, q0 : q0 + qn], start=True, stop=True,
                )
                nc.scalar.activation(
                    s["t2sb"][:, q0 : q0 + qn], tp[0:95, 0, 0:qn], Exp
                )
                ringA.dma_start(
                    t2d[hh][0:95, q0 : q0 + qn], s["t2sb"][0:95, q0 : q0 + qn]
                )
                (ba, bb) = PIECES[ci]
                nbl = bb - ba
                # rel_w (w-major): relw[j, (w,h)] = T2w[47-w+j, 48w+h]
                dstw = s["relw"][0:48, 48 * ba : 48 * bb].rearrange(
                    "p (w h) -> p w h", h=48
                )
                srcw = t2d[hh][47 - ba : 95 - ba, 48 * ba : 48 * bb].rearrange(
                    "j (w h) -> j w h", h=48
                )
                srcw.ap[1] = [-2256, nbl]
                ringB.dma_start(dstw, srcw)

            def tabh(hh, ci, cast_act=False):
                # T1h chunk: matmul -> cast -> DRAM store -> diagonal gather
                s = hs[hh]
                ringA = ringB = nc.sync if hh == 0 else nc.gpsimd
                (q0, qn) = QCHUNKS[ci]
                tp = ps_s.tile([128, 3, 512], F32, tag="s", name=f"tph{hh}_{ci}")
                nc.tensor.matmul(
                    tp[0:95, 0, 0:qn], rhv_sb[64:128, :],
                    s["rq"][64:128, q0 : q0 + qn], start=True, stop=True,
                )
                if cast_act:
                    nc.scalar.activation(
                        s["t1sb"][:, q0 : q0 + qn], tp[0:95, 0, 0:qn], Copy
                    )
                else:
                    nc.vector.tensor_copy(s["t1sb"][:, q0 : q0 + qn], tp[0:95, 0, 0:qn])
                ringA.dma_start(
                    t1d[hh][0:95, q0 : q0 + qn], s["t1sb"][0:95, q0 : q0 + qn]
                )
                (ba, bb) = PIECES[ci]
                nbl = bb - ba
                # rel_h: rq[j, (h,w)] = T1h[47-h+j, 48h+w], h in [ba, bb)
                dsth = s["rq"][0:48, 48 * ba : 48 * bb].rearrange(
                    "p (h w) -> p h w", w=48
                )
                srch = t1d[hh][47 - ba : 95 - ba, 48 * ba : 48 * bb].rearrange(
                    "j (h w) -> j h w", w=48
                )
                srch.ap[1] = [-2256, nbl]
                ringB.dma_start(dsth, srch)

            def relw_exp(hh, half):
                # un-permuting (w-major -> q-major) copy of the already-
                # exponentiated rel_w rows, on the DVE
                s = hs[hh]
                h0 = half * 24  # h-blocks 0:24/24:48 -> cols 0:1152/1152:2304
                co, cn = h0 * 48, 1152
                nc.vector.tensor_copy(
                    s["expw"][:, co : co + cn].rearrange("p (h w) -> p h w", w=48),
                    s["relw"][:, :].rearrange("p (w h) -> p h w", w=48)[:, h0 : h0 + 24, :],
                )

            def pats_copy(hh, co, cn):
                # 3 row-rotations (k-tile offsets 0/32/16) of exp_relw
                s = hs[hh]
                ndma = 0
                for j, off in enumerate((0, 32, 16)):
                    p = 0
                    while p < 128:
                        r0 = (p + off) % 48
                        n = min(48 - r0, 128 - p)
                        eng = (nc.sync, nc.gpsimd)[(hh + ndma) % 2]
                        eng.dma_start(
                            s["pats"][p : p + n, j, co : co + cn],
                            s["expw"][r0 : r0 + n, co : co + cn],
                        )
                        p += n
                        ndma += 1

            # ---- main loop machinery (shared across heads) ----
            # PV pipeline: each chunk's PVs run during the NEXT chunk, one
            # full q-block (all 18 k-tiles, sequential PSUM accumulation)
            # per score group. Interleaving accumulation regions within a
            # PSUM bank corrupts all but the last region, so each block's
            # 18-matmul accumulation must be contiguous in the PE stream.
            st = {"pend": []}

            def epilogue(e):
                hh, ci, q0, qn, o_ps = e["hh"], e["ci"], e["q0"], e["qn"], e["o_ps"]
                nb = qn // 128
                rect = epool.tile([128, 4], F32, tag="rect", name=f"rect{hh}_{ci}")
                ot = epool.tile([128, 256], F32, tag="ot", name=f"ot{hh}_{ci}")
                for b in range(nb):
                    nc.vector.reciprocal(
                        rect[:, b : b + 1],
                        o_ps[:, 128 * b + 64 : 128 * b + 65],
                    )
                    nc.vector.tensor_scalar_mul(
                        ot[:, 64 * b : 64 * (b + 1)],
                        o_ps[:, 128 * b : 128 * b + 64],
                        rect[:, b : b + 1],
                    )
                dmae = nc.sync if ci % 2 == 0 else nc.gpsimd
                r0 = hh * HW + q0
                dmae.dma_start(
                    out_t[r0 : r0 + qn, :].rearrange("(b p) d -> p b d", p=128),
                    ot[:, 0 : 64 * nb].rearrange("p (b d) -> p b d", d=64),
                )

            def pv_step(last_mm):
                if not st["pend"]:
                    return
                e = st["pend"][0]
                b, o_ps, p2s, vt = e["b"], e["o_ps"], e["p2s"], e["vt"]
                for kt in range(KT):
                    g, j = divmod(kt, 3)
                    pv = nc.tensor.matmul(
                        o_ps[:, 128 * b : 128 * b + 65],
                        p2s[g][:, j, 128 * b : 128 * b + 128],
                        vt[:, kt * 65 : (kt + 1) * 65],
                        start=(kt == 0), stop=(kt == KT - 1),
                    )
                    if last_mm is not None:
                        add_dep_helper(pv.ins, last_mm.ins, sync=False,
                                       reason="pv after score mms")
                e["b"] += 1
                if e["b"] == e["qn"] // 128:
                    epilogue(e)
                    st["pend"].pop(0)

            def chunk(hh, ci, inserts=None):
                s = hs[hh]
                (q0, qn) = QCHUNKS[ci]
                o_ps = ps_o.tile([128, 512], F32, tag="o", name=f"o{hh}_{ci}")
                p2s = []
                for g in range(NG):
                    if inserts:
                        inserts.pop(0)()
                    s_ps = ps_s.tile([128, 3, 512], F32, tag="s",
                                     name=f"s{hh}_{ci}_{g}")
                    last_mm = None
                    for j in range(3):
                        kt = 3 * g + j
                        last_mm = nc.tensor.matmul(
                            s_ps[:, j, 0:qn],
                            s["lhsT"][:, kt * 128 : (kt + 1) * 128],
                            s["rq"][:, q0 : q0 + qn],
                            start=True, stop=True,
                        )
                    pv_step(last_mm)
                    if SCH_EVERY and (ci * NG + g) % SCH_EVERY == 2:
                        # Schraudolph exp on DVE (every 5th group)
                        p1i = p1ipool.tile([128, 3, 512], I16, tag="p1i")
                        nc.vector.tensor_scalar(
                            p1i[:, :, 0:qn], s_ps[:, :, 0:qn],
                            SCH_C1, SCH_C2, MULT, ADD,
                        )
                        p1v = p1i.bitcast(BF16)
                    else:
                        p1 = p1pool.tile([128, 3, 512], BF16, tag="p1")
                        nc.scalar.activation(p1[:, :, 0:qn], s_ps[:, :, 0:qn], Exp)
                        p1v = p1
                    p2 = p2pool.tile([128, 3, 512], BF16, tag="p2")
                    nc.vector.tensor_mul(
                        p2[:, :, 0:qn], p1v[:, :, 0:qn],
                        s["pats"][:, :, q0 : q0 + qn],
                    )
                    p2s.append(p2)
                st["pend"].append(dict(hh=hh, ci=ci, q0=q0, qn=qn, o_ps=o_ps,
                                       p2s=p2s, vt=s["vt"], b=0))

            def finish_all():
                while st["pend"]:
                    pv_step(None)

            # ---- schedule ----
            ph1_tiles(0)
            ph1_tiles(1)
            # preload the ACT exp table while DMAs are in flight
            warm = epool.tile([1, 64], BF16, tag="warm")
            nc.scalar.activation(warm[0:1, 0:64], ones1[0:1, 0:64], Exp)
            # startup-critical loads only: first halves of Q layouts + the
            # first 3 k-tiles of the K stack; the rest streams in at slots
            # 0-1 so the rel-table stores aren't queued behind them.
            ph1_load_q_min(0)
            ph1_load_lhs(0, 0, 384)
            # prep chains: T1h pieces 0-1 gate the first score matmul (casts
            # on the idle ACT; Copy shares the Exp table set); the T2w chain
            # (casts on the idle DVE) gates the first p2 multiply, with a
            # full chunk of score-matmul runway before that.
            tabh(0, 0, cast_act=True)
            tabh(0, 1, cast_act=True)
            for c in range(5):
                tabw(0, c)
            relw_exp(0, 0)
            pats_copy(0, 0, 1152)

            nop = lambda: None
            ins0 = [
                # during head-0 chunk 0 (6 slots)
                lambda: (ph1_load_q_rest(0), ph1_load_lhs(0, 384, 1152),
                         ph1_load_vt(0)),
                lambda: (ph1_load_lhs(0, 1152, HW), ph1_load_q(1)),
                lambda: tabh(0, 2),
                lambda: (ph1_load_lhs(1), ph1_load_vt(1)),
                lambda: relw_exp(0, 1),
                lambda: pats_copy(0, 1152, 1152),
                # chunk 1
                lambda: tabw(1, 0),
                nop,
                lambda: tabw(1, 1),
                nop,
                lambda: tabw(1, 2),
                lambda: tabh(0, 3),
                # chunk 2
                lambda: tabw(1, 3),
                nop,
                lambda: tabw(1, 4),
                lambda: tabh(0, 4),
                lambda: relw_exp(1, 0),
                nop,
                # chunk 3
                lambda: pats_copy(1, 0, 1152),
                nop,
                lambda: tabh(1, 0),
                lambda: relw_exp(1, 1),
                nop,
                lambda: tabh(1, 1),
                # chunk 4
                lambda: pats_copy(1, 1152, 1152),
                nop,
                lambda: tabh(1, 2),
                nop,
                lambda: tabh(1, 3),
                nop,
            ]
            ins1 = [
                lambda: tabh(1, 4),
            ] + [nop] * 29

            for ci in range(5):
                chunk(0, ci, ins0)
            for ci in range(5):
                chunk(1, ci, ins1)
            finish_all()

    nc.compile()
    return nc


def _get_nc():
    global _NC
    if _NC is None:
        _NC = _build_nc()
    return _NC


def _host_prep(q, k, v, rel_pos_h, rel_pos_w):
    q2 = np.asarray(q, np.float32).reshape(HW, NH * DH)
    k2 = np.asarray(k, np.float32).reshape(HW, NH * DH)
    v2 = np.asarray(v, np.float32).reshape(HW, NH * DH)
    rph = np.asarray(rel_pos_h, np.float32)
    rpw = np.asarray(rel_pos_w, np.float32)

    ar = np.arange(48)
    # reversed rel tables, x8 cancels the 0.125 q scale
    rhv = np.ascontiguousarray((8.0 * rph[::-1]).T).astype(BF)   # (64, 95)
    rwv = np.ascontiguousarray((8.0 * rpw[::-1]).T).astype(BF)
    kk = np.arange(HW)
    eh = np.zeros((64, HW), np.float32)
    eh[:48] = kk[None, :] // 48 == ar[:, None]
    eh = eh.astype(BF)

    onecol = np.ones((HW, 1), np.float32)
    in_maps = []
    for c in range(N_CORES):
        sl = slice(c * 128, (c + 1) * 128)
        qs = (q2[:, sl].T * 0.125).astype(BF)                    # (128, HW)
        qw = np.ascontiguousarray(
            qs.reshape(128, 48, 48).transpose(0, 2, 1)
        ).reshape(128, HW)                                       # w-major cols
        ks = k2[:, sl].T.astype(BF)
        lhs_p = np.zeros((128, HPC, HW), BF)
        rqs_p = np.zeros((80, HPC, HW), BF)
        rqw_p = np.zeros((64, HPC, HW), BF)
        vparts = []
        for hh in range(HPC):
            r0, r1 = hh * 64, (hh + 1) * 64
            lhs_p[0:64, hh, :] = eh
            lhs_p[64:128, hh, :] = ks[r0:r1]
            rqs_p[16:80, hh, :] = qs[r0:r1]
            rqw_p[:, hh, :] = qw[r0:r1]
            vh = v2[:, c * 128 + hh * 64 : c * 128 + (hh + 1) * 64]
            va = np.concatenate([vh, onecol], axis=1)            # (HW, 65)
            vparts.append(va.reshape(KT, 128, 65).transpose(1, 0, 2).reshape(128, KT * 65))
        v_til = np.concatenate(vparts, axis=1).astype(BF)        # (128, 2*18*65)
        in_maps.append(
            dict(
                lhs_p=lhs_p.reshape(128, HPC * HW),
                rqs_p=rqs_p.reshape(80, HPC * HW),
                rqw_p=rqw_p.reshape(64, HPC * HW),
                v_til=v_til, rhv=rhv, rwv=rwv,
            )
        )
    return in_maps


def _assemble(results):
    # out_t per core: [HPC*HW, 64] f32, head hh in rows hh*HW:(hh+1)*HW
    full = np.empty((HW, NH * DH), np.float32)
    for c, r in enumerate(results):
        o = np.asarray(r["out_t"], np.float32)
        for hh in range(HPC):
            full[:, c * 128 + hh * 64 : c * 128 + (hh + 1) * 64] = \
                o[hh * HW : (hh + 1) * HW, :]
    return full.reshape(1, H, W, NH * DH)


def kernel(q, k, v, rel_pos_h, rel_pos_w):
    nc = _get_nc()
    in_maps = _host_prep(q, k, v, rel_pos_h, rel_pos_w)
    res = run_bass_kernel_spmd(nc, in_maps, core_ids=list(range(N_CORES)))
    return _assemble(res.results)


# revision 41
# speedup vs baseline: 1.0222x; 1.0148x over previous
"""Trainium2 Bass kernel for SAM-style decomposed rel-pos attention (v6).

Problem: B=1, HW=2304 (48x48), NH=16 heads, DH=64, D=1024, f32 in/out.
  attn = softmax(q*scale @ k^T + rel_h[qh,kh] + rel_w[qw,kw]); out = attn @ v

Strategy (8 NeuronCores, SPMD): 2 heads per core. Key ideas:
- rel_h folded into the score matmul (one-hot Eh rows + K^T stationary;
  gathered rel_h rows + Q^T moving) -> 18 score matmuls per (head, chunk).
- rel_w applied multiplicatively after exp: P = exp(S_qk+relh) * Ew with
  only 3 row-rotations of exp_relw -> [128, 3, HW] patterns tile.
- Diagonal gathers via scratch-DRAM roundtrip with a -2256-stride AP.
- exp on ScalarE in [128, 3, qn] groups; PV matmuls lag PV_LAG groups.

v6 changes:
- TRANSPOSED PV: P tiles are the stationary operand ([128k, 128q] slices),
  V the moving ([128k, 65]); output lands [q-partition, dh] in one PSUM
  bank per chunk, accumulated over all 18 k-tiles. The ones-column
  denominator arrives per-partition, so softmax division is reciprocal +
  a per-partition tensor_scalar multiply: the old den-transpose DMAs,
  reciprocal-broadcast matmul, and deferred-epilogue machinery all go.
  PE cost is neutral: 324 LDWEIGHTS-paced slots x 128 cycles == the old
  column-paced PV stream.
- Schraudolph exp on DVE for every 5th group: P = bitcast_bf16(int16(
  S*(128/ln2) + (127*128 - 7))), ~1.8% rms on those tiles (validated
  0.8% end-to-end), relieving the ScalarE exp floor (the true wall).
- Startup pipelining: rqs/rqw/lhsT split across both DMA rings up front;
  vt + head-1 loads stream in mid-loop; T1h pieces 0-1 gate the first
  score matmul; PV lag 6 on chunk 0 gives the T2w->exp->pats chain
  score-matmul runway. ACT exp table preloaded via a dummy activation.
"""

import sys

sys.path.insert(0, "/opt/trn_rl_repo")

import numpy as np
import ml_dtypes

from concourse import bacc, mybir, tile
from concourse.tile import add_dep_helper
from concourse.bass_utils import run_bass_kernel_spmd

BF16 = mybir.dt.bfloat16
F32 = mybir.dt.float32
I16 = mybir.dt.int16
BF = ml_dtypes.bfloat16

H = 48
W = 48
HW = H * W          # 2304
DH = 64
NH = 16
N_CORES = 8
HPC = 2             # heads per core
KT = HW // 128      # 18 k tiles
QCHUNKS = [(0, 512), (512, 512), (1024, 512), (1536, 512), (2048, 256)]
NG = KT // 3        # 6 groups of 3 k-tiles
PV_LAG = 4          # PV runs this many groups behind the score matmuls

# Schraudolph bf16 exp: bitcast(int16(x*128/ln2 + 127*128 - C)), C=7
SCH_C1 = 128.0 / float(np.log(2.0))
SCH_C2 = 127.0 * 128.0 - 7.0
SCH_EVERY = 6       # 0 = disabled; N = every Nth group on DVE

# gather piece ranges: piece c covers blocks [lo, hi) whose table cols
# fit inside table-store chunks 0..c (QCHUNKS cols)
PIECES = [(0, 10), (10, 21), (21, 32), (32, 42), (42, 48)]

_NC = None


def _build_nc():
    nc = bacc.Bacc(None, target_bir_lowering=False)

    lhs_p = nc.dram_tensor("lhs_p", [128, HPC * HW], BF16, kind="ExternalInput")
    rqs_p = nc.dram_tensor("rqs_p", [80, HPC * HW], BF16, kind="ExternalInput")
    rqw_p = nc.dram_tensor("rqw_p", [64, HPC * HW], BF16, kind="ExternalInput")
    v_til = nc.dram_tensor("v_til", [128, HPC * KT * 65], BF16, kind="ExternalInput")
    rhv = nc.dram_tensor("rhv", [64, 95], BF16, kind="ExternalInput")
    rwv = nc.dram_tensor("rwv", [64, 95], BF16, kind="ExternalInput")
    out_t = nc.dram_tensor("out_t", [HPC * HW, 64], F32, kind="ExternalOutput")
    t1d = [nc.dram_tensor(f"t1d{h}", [95, HW], BF16, kind="Internal") for h in range(HPC)]
    t2d = [nc.dram_tensor(f"t2d{h}", [95, HW], BF16, kind="Internal") for h in range(HPC)]

    Exp = mybir.ActivationFunctionType.Exp
    Copy = mybir.ActivationFunctionType.Copy
    MULT = mybir.AluOpType.mult
    ADD = mybir.AluOpType.add

    with tile.TileContext(nc) as tc:
        with (
            tc.tile_pool(name="const", bufs=1) as cpool,
            tc.tile_pool(name="stack", bufs=2) as spool,
            tc.tile_pool(name="p1t", bufs=4) as p1pool,
            tc.tile_pool(name="p1i", bufs=2) as p1ipool,
            tc.tile_pool(name="p2t", bufs=10) as p2pool,
            tc.tile_pool(name="epil", bufs=3) as epool,
            tc.tile_pool(name="ps_s", bufs=2, space="PSUM") as ps_s,
            tc.tile_pool(name="ps_o", bufs=2, space="PSUM") as ps_o,
        ):
            # rel tables on both partition halves: rows 0:64 feed the T2w
            # matmuls (rqw lives on partitions 0:64), rows 64:128 feed T1h
            # (q-major Q^T lives on rq partitions 64:128).
            rhv_sb = cpool.tile([128, 95], BF16, tag="rhv")
            rwv_sb = cpool.tile([128, 95], BF16, tag="rwv")
            ones1 = cpool.tile([1, 64], BF16, tag="ones1")
            nc.sync.dma_start(rwv_sb[0:64, :], rwv[:, :])
            nc.gpsimd.dma_start(rhv_sb[64:128, :], rhv[:, :])
            nc.gpsimd.memset(ones1[:], 1.0)

            hs = [dict() for _ in range(HPC)]

            def ph1_tiles(hh):
                s = hs[hh]
                s["lhsT"] = spool.tile([128, HW], BF16, tag="lhsT", name=f"lhsT{hh}")
                s["rq"] = spool.tile([128, HW], BF16, tag="rq", name=f"rq{hh}")
                s["rqw"] = spool.tile([64, HW], BF16, tag="rqw", name=f"rqw{hh}")
                s["vt"] = spool.tile([128, KT * 65], BF16, tag="vt", name=f"vt{hh}")
                s["t1sb"] = spool.tile([95, HW], BF16, tag="t1sb", name=f"t1sb{hh}")
                s["t2sb"] = spool.tile([95, HW], BF16, tag="t2sb", name=f"t2sb{hh}")
                s["relw"] = spool.tile([48, HW], BF16, tag="relw", name=f"relw{hh}")
                s["expw"] = spool.tile([48, HW], BF16, tag="expw", name=f"expw{hh}")
                s["pats"] = spool.tile([128, 3, HW], BF16, tag="pats", name=f"pats{hh}")

            def ph1_load_q(hh):
                # rq (q-major Q^T + zero rows) and rqw (w-major): the prep
                # critical path. Halves split across both rings.
                s = hs[hh]
                o = hh * HW
                nc.sync.dma_start(s["rq"][48:128, 0:1152], rqs_p[:, o : o + 1152])
                nc.gpsimd.dma_start(
                    s["rq"][48:128, 1152:HW], rqs_p[:, o + 1152 : o + HW]
                )
                nc.sync.dma_start(s["rqw"][:, 0:1152], rqw_p[:, o : o + 1152])
                nc.gpsimd.dma_start(s["rqw"][:, 1152:HW], rqw_p[:, o + 1152 : o + HW])

            def ph1_load_q_min(hh):
                # startup-minimal: only the halves the first tab matmuls and
                # first two chunks touch; the rqs second half comes at slot 0
                s = hs[hh]
                o = hh * HW
                nc.sync.dma_start(s["rq"][48:128, 0:1152], rqs_p[:, o : o + 1152])
                nc.sync.dma_start(s["rqw"][:, 0:1152], rqw_p[:, o : o + 1152])
                nc.gpsimd.dma_start(s["rqw"][:, 1152:HW], rqw_p[:, o + 1152 : o + HW])

            def ph1_load_q_rest(hh):
                s = hs[hh]
                o = hh * HW
                nc.gpsimd.dma_start(
                    s["rq"][48:128, 1152:HW], rqs_p[:, o + 1152 : o + HW]
                )

            def ph1_load_lhs(hh, c0=0, c1=HW):
                s = hs[hh]
                o = hh * HW
                if c1 - c0 > 1152:
                    nc.sync.dma_start(
                        s["lhsT"][:, c0:1152], lhs_p[:, o + c0 : o + 1152]
                    )
                    nc.gpsimd.dma_start(
                        s["lhsT"][:, 1152:c1], lhs_p[:, o + 1152 : o + c1]
                    )
                else:
                    nc.sync.dma_start(s["lhsT"][:, c0:c1], lhs_p[:, o + c0 : o + c1])

            def ph1_load_vt(hh):
                s = hs[hh]
                eng = nc.gpsimd if hh == 0 else nc.sync
                eng.dma_start(s["vt"][:, :], v_til[:, hh * KT * 65 : (hh + 1) * KT * 65])

            def tabw(hh, ci, cast_act=False):
                # T2w chunk: matmul -> cast -> DRAM store -> diagonal gather.
                # Store and gather share a ring: queue FIFO ordering makes
                # the store->gather dependency nearly latency-free.
                s = hs[hh]
                ringA = ringB = nc.gpsimd if hh == 0 else nc.sync
                (q0, qn) = QCHUNKS[ci]
                tp = ps_s.tile([128, 3, 512], F32, tag="s", name=f"tpw{hh}_{ci}")
                nc.tensor.matmul(
                    tp[0:95, 0, 0:qn], rwv_sb[0:64, :],
                    s["rqw"][:, q0 : q0 + qn], start=True, stop=True,
                )
                if cast_act:
                    nc.scalar.activation(
                        s["t2sb"][:, q0 : q0 + qn], tp[0:95, 0, 0:qn], Copy
                    )
                else:
                    nc.vector.tensor_copy(s["t2sb"][:, q0 : q0 + qn], tp[0:95, 0, 0:qn])
                ringA.dma_start(
                    t2d[hh][0:95, q0 : q0 + qn], s["t2sb"][0:95, q0 : q0 + qn]
                )
                (ba, bb) = PIECES[ci]
                nbl = bb - ba
                # rel_w (w-major): relw[j, (w,h)] = T2w[47-w+j, 48w+h]
                dstw = s["relw"][0:48, 48 * ba : 48 * bb].rearrange(
                    "p (w h) -> p w h", h=48
                )
                srcw = t2d[hh][47 - ba : 95 - ba, 48 * ba : 48 * bb].rearrange(
                    "j (w h) -> j w h", h=48
                )
                srcw.ap[1] = [-2256, nbl]
                ringB.dma_start(dstw, srcw)

            def tabh(hh, ci, cast_act=False):
                # T1h chunk: matmul -> cast -> DRAM store -> diagonal gather
                s = hs[hh]
                ringA = ringB = nc.sync if hh == 0 else nc.gpsimd
                (q0, qn) = QCHUNKS[ci]
                tp = ps_s.tile([128, 3, 512], F32, tag="s", name=f"tph{hh}_{ci}")
                nc.tensor.matmul(
                    tp[0:95, 0, 0:qn], rhv_sb[64:128, :],
                    s["rq"][64:128, q0 : q0 + qn], start=True, stop=True,
                )
                if cast_act:
                    nc.scalar.activation(
                        s["t1sb"][:, q0 : q0 + qn], tp[0:95, 0, 0:qn], Copy
                    )
                else:
                    nc.vector.tensor_copy(s["t1sb"][:, q0 : q0 + qn], tp[0:95, 0, 0:qn])
                ringA.dma_start(
                    t1d[hh][0:95, q0 : q0 + qn], s["t1sb"][0:95, q0 : q0 + qn]
                )
                (ba, bb) = PIECES[ci]
                nbl = bb - ba
                # rel_h: rq[j, (h,w)] = T1h[47-h+j, 48h+w], h in [ba, bb)
                dsth = s["rq"][0:48, 48 * ba : 48 * bb].rearrange(
                    "p (h w) -> p h w", w=48
                )
                srch = t1d[hh][47 - ba : 95 - ba, 48 * ba : 48 * bb].rearrange(
                    "j (h w) -> j h w", w=48
                )
                srch.ap[1] = [-2256, nbl]
                ringB.dma_start(dsth, srch)

            def relw_exp(hh, half):
                # exp with an un-permuting (w-major -> q-major) input AP
                s = hs[hh]
                h0 = half * 24  # h-blocks 0:24/24:48 -> cols 0:1152/1152:2304
                co, cn = h0 * 48, 1152
                nc.scalar.activation(
                    s["expw"][:, co : co + cn].rearrange("p (h w) -> p h w", w=48),
                    s["relw"][:, :].rearrange("p (w h) -> p h w", w=48)[:, h0 : h0 + 24, :],
                    Exp,
                )

            def pats_copy(hh, co, cn):
                # 3 row-rotations (k-tile offsets 0/32/16) of exp_relw
                s = hs[hh]
                ndma = 0
                for j, off in enumerate((0, 32, 16)):
                    p = 0
                    while p < 128:
                        r0 = (p + off) % 48
                        n = min(48 - r0, 128 - p)
                        eng = (nc.sync, nc.gpsimd)[(hh + ndma) % 2]
                        eng.dma_start(
                            s["pats"][p : p + n, j, co : co + cn],
                            s["expw"][r0 : r0 + n, co : co + cn],
                        )
                        p += n
                        ndma += 1

            # ---- main loop machinery (shared across heads) ----
            # PV pipeline: each chunk's PVs run during the NEXT chunk, one
            # full q-block (all 18 k-tiles, sequential PSUM accumulation)
            # per score group. Interleaving accumulation regions within a
            # PSUM bank corrupts all but the last region, so each block's
            # 18-matmul accumulation must be contiguous in the PE stream.
            st = {"pend": []}

            def epilogue(e):
                hh, ci, q0, qn, o_ps = e["hh"], e["ci"], e["q0"], e["qn"], e["o_ps"]
                nb = qn // 128
                rect = epool.tile([128, 4], F32, tag="rect", name=f"rect{hh}_{ci}")
                ot = epool.tile([128, 256], F32, tag="ot", name=f"ot{hh}_{ci}")
                for b in range(nb):
                    nc.vector.reciprocal(
                        rect[:, b : b + 1],
                        o_ps[:, 128 * b + 64 : 128 * b + 65],
                    )
                    nc.vector.tensor_scalar_mul(
                        ot[:, 64 * b : 64 * (b + 1)],
                        o_ps[:, 128 * b : 128 * b + 64],
                        rect[:, b : b + 1],
                    )
                dmae = nc.sync if ci % 2 == 0 else nc.gpsimd
                r0 = hh * HW + q0
                dmae.dma_start(
                    out_t[r0 : r0 + qn, :].rearrange("(b p) d -> p b d", p=128),
                    ot[:, 0 : 64 * nb].rearrange("p (b d) -> p b d", d=64),
                )

            def pv_step(last_mm):
                if not st["pend"]:
                    return
                e = st["pend"][0]
                b, o_ps, p2s, vt = e["b"], e["o_ps"], e["p2s"], e["vt"]
                for kt in range(KT):
                    g, j = divmod(kt, 3)
                    pv = nc.tensor.matmul(
                        o_ps[:, 128 * b : 128 * b + 65],
                        p2s[g][:, j, 128 * b : 128 * b + 128],
                        vt[:, kt * 65 : (kt + 1) * 65],
                        start=(kt == 0), stop=(kt == KT - 1),
                    )
                    if last_mm is not None:
                        add_dep_helper(pv.ins, last_mm.ins, sync=False,
                                       reason="pv after score mms")
                e["b"] += 1
                if e["b"] == e["qn"] // 128:
                    epilogue(e)
                    st["pend"].pop(0)

            def chunk(hh, ci, inserts=None):
                s = hs[hh]
                (q0, qn) = QCHUNKS[ci]
                o_ps = ps_o.tile([128, 512], F32, tag="o", name=f"o{hh}_{ci}")
                p2s = []
                for g in range(NG):
                    if inserts:
                        inserts.pop(0)()
                    s_ps = ps_s.tile([128, 3, 512], F32, tag="s",
                                     name=f"s{hh}_{ci}_{g}")
                    last_mm = None
                    for j in range(3):
                        kt = 3 * g + j
                        last_mm = nc.tensor.matmul(
                            s_ps[:, j, 0:qn],
                            s["lhsT"][:, kt * 128 : (kt + 1) * 128],
                            s["rq"][:, q0 : q0 + qn],
                            start=True, stop=True,
                        )
                    pv_step(last_mm)
                    if SCH_EVERY and (ci * NG + g) % SCH_EVERY == 2:
                        # Schraudolph exp on DVE (every 5th group)
                        p1i = p1ipool.tile([128, 3, 512], I16, tag="p1i")
                        nc.vector.tensor_scalar(
                            p1i[:, :, 0:qn], s_ps[:, :, 0:qn],
                            SCH_C1, SCH_C2, MULT, ADD,
                        )
                        p1v = p1i.bitcast(BF16)
                    else:
                        p1 = p1pool.tile([128, 3, 512], BF16, tag="p1")
                        nc.scalar.activation(p1[:, :, 0:qn], s_ps[:, :, 0:qn], Exp)
                        p1v = p1
                    p2 = p2pool.tile([128, 3, 512], BF16, tag="p2")
                    nc.vector.tensor_mul(
                        p2[:, :, 0:qn], p1v[:, :, 0:qn],
                        s["pats"][:, :, q0 : q0 + qn],
                    )
                    p2s.append(p2)
                st["pend"].append(dict(hh=hh, ci=ci, q0=q0, qn=qn, o_ps=o_ps,
                                       p2s=p2s, vt=s["vt"], b=0))

            def finish_all():
                while st["pend"]:
                    pv_step(None)

            # ---- schedule ----
            ph1_tiles(0)
            ph1_tiles(1)
            # preload the ACT exp table while DMAs are in flight
            warm = epool.tile([1, 64], BF16, tag="warm")
            nc.scalar.activation(warm[0:1, 0:64], ones1[0:1, 0:64], Exp)
            # startup-critical loads only: first halves of Q layouts + the
            # first 3 k-tiles of the K stack; the rest streams in at slots
            # 0-1 so the rel-table stores aren't queued behind them.
            ph1_load_q_min(0)
            ph1_load_lhs(0, 0, 384)
            # prep chains: T1h pieces 0-1 gate the first score matmul (casts
            # on the idle ACT; Copy shares the Exp table set); the T2w chain
            # (casts on the idle DVE) gates the first p2 multiply, with a
            # full chunk of score-matmul runway before that.
            tabh(0, 0, cast_act=True)
            tabh(0, 1, cast_act=True)
            for c in range(5):
                tabw(0, c)
            relw_exp(0, 0)
            pats_copy(0, 0, 1152)

            nop = lambda: None
            ins0 = [
                # during head-0 chunk 0 (6 slots)
                lambda: (ph1_load_q_rest(0), ph1_load_lhs(0, 384, 1152),
                         ph1_load_vt(0)),
                lambda: (ph1_load_lhs(0, 1152, HW), ph1_load_q(1)),
                lambda: tabh(0, 2),
                lambda: (ph1_load_lhs(1), ph1_load_vt(1)),
                lambda: relw_exp(0, 1),
                lambda: pats_copy(0, 1152, 1152),
                # chunk 1
                lambda: tabw(1, 0),
                nop,
                lambda: tabw(1, 1),
                nop,
                lambda: tabw(1, 2),
                lambda: tabh(0, 3),
                # chunk 2
                lambda: tabw(1, 3),
                nop,
                lambda: tabw(1, 4),
                lambda: tabh(0, 4),
                lambda: relw_exp(1, 0),
                nop,
                # chunk 3
                lambda: pats_copy(1, 0, 1152),
                nop,
                lambda: tabh(1, 0),
                lambda: relw_exp(1, 1),
                nop,
                lambda: tabh(1, 1),
                # chunk 4
                lambda: pats_copy(1, 1152, 1152),
                nop,
                lambda: tabh(1, 2),
                nop,
                lambda: tabh(1, 3),
                nop,
            ]
            ins1 = [
                lambda: tabh(1, 4),
            ] + [nop] * 29

            for ci in range(5):
                chunk(0, ci, ins0)
            for ci in range(5):
                chunk(1, ci, ins1)
            finish_all()

    nc.compile()
    return nc


def _get_nc():
    global _NC
    if _NC is None:
        _NC = _build_nc()
    return _NC


def _host_prep(q, k, v, rel_pos_h, rel_pos_w):
    q2 = np.asarray(q, np.float32).reshape(HW, NH * DH)
    k2 = np.asarray(k, np.float32).reshape(HW, NH * DH)
    v2 = np.asarray(v, np.float32).reshape(HW, NH * DH)
    rph = np.asarray(rel_pos_h, np.float32)
    rpw = np.asarray(rel_pos_w, np.float32)

    ar = np.arange(48)
    # reversed rel tables, x8 cancels the 0.125 q scale
    rhv = np.ascontiguousarray((8.0 * rph[::-1]).T).astype(BF)   # (64, 95)
    rwv = np.ascontiguousarray((8.0 * rpw[::-1]).T).astype(BF)
    kk = np.arange(HW)
    eh = np.zeros((64, HW), np.float32)
    eh[:48] = kk[None, :] // 48 == ar[:, None]
    eh = eh.astype(BF)

    onecol = np.ones((HW, 1), np.float32)
    in_maps = []
    for c in range(N_CORES):
        sl = slice(c * 128, (c + 1) * 128)
        qs = (q2[:, sl].T * 0.125).astype(BF)                    # (128, HW)
        qw = np.ascontiguousarray(
            qs.reshape(128, 48, 48).transpose(0, 2, 1)
        ).reshape(128, HW)                                       # w-major cols
        ks = k2[:, sl].T.astype(BF)
        lhs_p = np.zeros((128, HPC, HW), BF)
        rqs_p = np.zeros((80, HPC, HW), BF)
        rqw_p = np.zeros((64, HPC, HW), BF)
        vparts = []
        for hh in range(HPC):
            r0, r1 = hh * 64, (hh + 1) * 64
            lhs_p[0:64, hh, :] = eh
            lhs_p[64:128, hh, :] = ks[r0:r1]
            rqs_p[16:80, hh, :] = qs[r0:r1]
            rqw_p[:, hh, :] = qw[r0:r1]
            vh = v2[:, c * 128 + hh * 64 : c * 128 + (hh + 1) * 64]
            va = np.concatenate([vh, onecol], axis=1)            # (HW, 65)
            vparts.append(va.reshape(KT, 128, 65).transpose(1, 0, 2).reshape(128, KT * 65))
        v_til = np.concatenate(vparts, axis=1).astype(BF)        # (128, 2*18*65)
        in_maps.append(
            dict(
                lhs_p=lhs_p.reshape(128, HPC * HW),
                rqs_p=rqs_p.reshape(80, HPC * HW),
                rqw_p=rqw_p.reshape(64, HPC * HW),
                v_til=v_til, rhv=rhv, rwv=rwv,
            )
        )
    return in_maps


def _assemble(results):
    # out_t per core: [HPC*HW, 64] f32, head hh in rows hh*HW:(hh+1)*HW
    full = np.empty((HW, NH * DH), np.float32)
    for c, r in enumerate(results):
        o = np.asarray(r["out_t"], np.float32)
        for hh in range(HPC):
            full[:, c * 128 + hh * 64 : c * 128 + (hh + 1) * 64] = \
                o[hh * HW : (hh + 1) * HW, :]
    return full.reshape(1, H, W, NH * DH)


def kernel(q, k, v, rel_pos_h, rel_pos_w):
    nc = _get_nc()
    in_maps = _host_prep(q, k, v, rel_pos_h, rel_pos_w)
    res = run_bass_kernel_spmd(nc, in_maps, core_ids=list(range(N_CORES)))
    return _assemble(res.results)


# revision 42
# speedup vs baseline: 1.0229x; 1.0007x over previous
"""Trainium2 Bass kernel for SAM-style decomposed rel-pos attention (v6).

Problem: B=1, HW=2304 (48x48), NH=16 heads, DH=64, D=1024, f32 in/out.
  attn = softmax(q*scale @ k^T + rel_h[qh,kh] + rel_w[qw,kw]); out = attn @ v

Strategy (8 NeuronCores, SPMD): 2 heads per core. Key ideas:
- rel_h folded into the score matmul (one-hot Eh rows + K^T stationary;
  gathered rel_h rows + Q^T moving) -> 18 score matmuls per (head, chunk).
- rel_w applied multiplicatively after exp: P = exp(S_qk+relh) * Ew with
  only 3 row-rotations of exp_relw -> [128, 3, HW] patterns tile.
- Diagonal gathers via scratch-DRAM roundtrip with a -2256-stride AP.
- exp on ScalarE in [128, 3, qn] groups; PV matmuls lag PV_LAG groups.

v6 changes:
- TRANSPOSED PV: P tiles are the stationary operand ([128k, 128q] slices),
  V the moving ([128k, 65]); output lands [q-partition, dh] in one PSUM
  bank per chunk, accumulated over all 18 k-tiles. The ones-column
  denominator arrives per-partition, so softmax division is reciprocal +
  a per-partition tensor_scalar multiply: the old den-transpose DMAs,
  reciprocal-broadcast matmul, and deferred-epilogue machinery all go.
  PE cost is neutral: 324 LDWEIGHTS-paced slots x 128 cycles == the old
  column-paced PV stream.
- Schraudolph exp on DVE for every 5th group: P = bitcast_bf16(int16(
  S*(128/ln2) + (127*128 - 7))), ~1.8% rms on those tiles (validated
  0.8% end-to-end), relieving the ScalarE exp floor (the true wall).
- Startup pipelining: rqs/rqw/lhsT split across both DMA rings up front;
  vt + head-1 loads stream in mid-loop; T1h pieces 0-1 gate the first
  score matmul; PV lag 6 on chunk 0 gives the T2w->exp->pats chain
  score-matmul runway. ACT exp table preloaded via a dummy activation.
"""

import sys

sys.path.insert(0, "/opt/trn_rl_repo")

import numpy as np
import ml_dtypes

from concourse import bacc, mybir, tile
from concourse.tile import add_dep_helper
from concourse.bass_utils import run_bass_kernel_spmd

BF16 = mybir.dt.bfloat16
F32 = mybir.dt.float32
I16 = mybir.dt.int16
BF = ml_dtypes.bfloat16

H = 48
W = 48
HW = H * W          # 2304
DH = 64
NH = 16
N_CORES = 8
HPC = 2             # heads per core
KT = HW // 128      # 18 k tiles
QCHUNKS = [(0, 512), (512, 512), (1024, 512), (1536, 512), (2048, 256)]
NG = KT // 3        # 6 groups of 3 k-tiles
PV_LAG = 4          # PV runs this many groups behind the score matmuls

# Schraudolph bf16 exp: bitcast(int16(x*128/ln2 + 127*128 - C)), C=7
SCH_C1 = 128.0 / float(np.log(2.0))
SCH_C2 = 127.0 * 128.0 - 7.0
SCH_EVERY = 6       # 0 = disabled; N = every Nth group on DVE

# gather piece ranges: piece c covers blocks [lo, hi) whose table cols
# fit inside table-store chunks 0..c (QCHUNKS cols)
PIECES = [(0, 10), (10, 21), (21, 32), (32, 42), (42, 48)]

_NC = None


def _build_nc():
    nc = bacc.Bacc(None, target_bir_lowering=False)

    lhs_p = nc.dram_tensor("lhs_p", [128, HPC * HW], BF16, kind="ExternalInput")
    rqs_p = nc.dram_tensor("rqs_p", [80, HPC * HW], BF16, kind="ExternalInput")
    rqw_p = nc.dram_tensor("rqw_p", [64, HPC * HW], BF16, kind="ExternalInput")
    v_til = nc.dram_tensor("v_til", [128, HPC * KT * 65], BF16, kind="ExternalInput")
    rhv = nc.dram_tensor("rhv", [64, 95], BF16, kind="ExternalInput")
    rwv = nc.dram_tensor("rwv", [64, 95], BF16, kind="ExternalInput")
    out_t = nc.dram_tensor("out_t", [HPC * HW, 64], F32, kind="ExternalOutput")
    t1d = [nc.dram_tensor(f"t1d{h}", [95, HW], BF16, kind="Internal") for h in range(HPC)]
    t2d = [nc.dram_tensor(f"t2d{h}", [95, HW], BF16, kind="Internal") for h in range(HPC)]

    Exp = mybir.ActivationFunctionType.Exp
    Copy = mybir.ActivationFunctionType.Copy
    MULT = mybir.AluOpType.mult
    ADD = mybir.AluOpType.add

    with tile.TileContext(nc) as tc:
        with (
            tc.tile_pool(name="const", bufs=1) as cpool,
            tc.tile_pool(name="stack", bufs=2) as spool,
            tc.tile_pool(name="p1t", bufs=4) as p1pool,
            tc.tile_pool(name="p1i", bufs=2) as p1ipool,
            tc.tile_pool(name="p2t", bufs=10) as p2pool,
            tc.tile_pool(name="epil", bufs=3) as epool,
            tc.tile_pool(name="ps_s", bufs=2, space="PSUM") as ps_s,
            tc.tile_pool(name="ps_o", bufs=2, space="PSUM") as ps_o,
        ):
            # rel tables on both partition halves: rows 0:64 feed the T2w
            # matmuls (rqw lives on partitions 0:64), rows 64:128 feed T1h
            # (q-major Q^T lives on rq partitions 64:128).
            rhv_sb = cpool.tile([128, 95], BF16, tag="rhv")
            rwv_sb = cpool.tile([128, 95], BF16, tag="rwv")
            ones1 = cpool.tile([1, 64], BF16, tag="ones1")
            nc.sync.dma_start(rwv_sb[0:64, :], rwv[:, :])
            nc.gpsimd.dma_start(rhv_sb[64:128, :], rhv[:, :])
            nc.gpsimd.memset(ones1[:], 1.0)

            hs = [dict() for _ in range(HPC)]

            def ph1_tiles(hh):
                s = hs[hh]
                s["lhsT"] = spool.tile([128, HW], BF16, tag="lhsT", name=f"lhsT{hh}")
                s["rq"] = spool.tile([128, HW], BF16, tag="rq", name=f"rq{hh}")
                s["rqw"] = spool.tile([64, HW], BF16, tag="rqw", name=f"rqw{hh}")
                s["vt"] = spool.tile([128, KT * 65], BF16, tag="vt", name=f"vt{hh}")
                s["t1sb"] = spool.tile([95, HW], BF16, tag="t1sb", name=f"t1sb{hh}")
                s["t2sb"] = spool.tile([95, HW], BF16, tag="t2sb", name=f"t2sb{hh}")
                s["relw"] = spool.tile([48, HW], BF16, tag="relw", name=f"relw{hh}")
                s["expw"] = spool.tile([48, HW], BF16, tag="expw", name=f"expw{hh}")
                s["pats"] = spool.tile([128, 3, HW], BF16, tag="pats", name=f"pats{hh}")

            def ph1_load_q(hh):
                # rq (q-major Q^T + zero rows) and rqw (w-major): the prep
                # critical path. Halves split across both rings.
                s = hs[hh]
                o = hh * HW
                nc.sync.dma_start(s["rq"][48:128, 0:1152], rqs_p[:, o : o + 1152])
                nc.gpsimd.dma_start(
                    s["rq"][48:128, 1152:HW], rqs_p[:, o + 1152 : o + HW]
                )
                nc.sync.dma_start(s["rqw"][:, 0:1152], rqw_p[:, o : o + 1152])
                nc.gpsimd.dma_start(s["rqw"][:, 1152:HW], rqw_p[:, o + 1152 : o + HW])

            def ph1_load_q_min(hh):
                # startup-minimal: only the halves the first tab matmuls and
                # first two chunks touch; the rqs second half comes at slot 0
                s = hs[hh]
                o = hh * HW
                nc.sync.dma_start(s["rq"][48:128, 0:1152], rqs_p[:, o : o + 1152])
                nc.sync.dma_start(s["rqw"][:, 0:1152], rqw_p[:, o : o + 1152])
                nc.gpsimd.dma_start(s["rqw"][:, 1152:HW], rqw_p[:, o + 1152 : o + HW])

            def ph1_load_q_rest(hh):
                s = hs[hh]
                o = hh * HW
                nc.gpsimd.dma_start(
                    s["rq"][48:128, 1152:HW], rqs_p[:, o + 1152 : o + HW]
                )

            def ph1_load_lhs(hh, c0=0, c1=HW):
                s = hs[hh]
                o = hh * HW
                if c1 - c0 > 1152:
                    nc.sync.dma_start(
                        s["lhsT"][:, c0:1152], lhs_p[:, o + c0 : o + 1152]
                    )
                    nc.gpsimd.dma_start(
                        s["lhsT"][:, 1152:c1], lhs_p[:, o + 1152 : o + c1]
                    )
                else:
                    nc.sync.dma_start(s["lhsT"][:, c0:c1], lhs_p[:, o + c0 : o + c1])

            def ph1_load_vt(hh):
                s = hs[hh]
                eng = nc.gpsimd if hh == 0 else nc.sync
                eng.dma_start(s["vt"][:, :], v_til[:, hh * KT * 65 : (hh + 1) * KT * 65])

            def tabw(hh, ci):
                # T2w chunk: matmul -> EXP-cast -> DRAM store -> diagonal
                # gather. The table is exponentiated at the PSUM cast (exp
                # commutes with the gather/permute), so the un-permuting
                # pass later is a plain copy instead of a strided exp.
                # Store and gather share a ring: queue FIFO ordering makes
                # the store->gather dependency nearly latency-free.
                s = hs[hh]
                ringA = ringB = nc.gpsimd if hh == 0 else nc.sync
                (q0, qn) = QCHUNKS[ci]
                tp = ps_s.tile([128, 3, 512], F32, tag="s", name=f"tpw{hh}_{ci}")
                nc.tensor.matmul(
                    tp[0:95, 0, 0:qn], rwv_sb[0:64, :],
                    s["rqw"][:, q0 : q0 + qn], start=True, stop=True,
                )
                nc.scalar.activation(
                    s["t2sb"][:, q0 : q0 + qn], tp[0:95, 0, 0:qn], Exp
                )
                ringA.dma_start(
                    t2d[hh][0:95, q0 : q0 + qn], s["t2sb"][0:95, q0 : q0 + qn]
                )
                (ba, bb) = PIECES[ci]
                nbl = bb - ba
                # rel_w (w-major): relw[j, (w,h)] = T2w[47-w+j, 48w+h]
                dstw = s["relw"][0:48, 48 * ba : 48 * bb].rearrange(
                    "p (w h) -> p w h", h=48
                )
                srcw = t2d[hh][47 - ba : 95 - ba, 48 * ba : 48 * bb].rearrange(
                    "j (w h) -> j w h", h=48
                )
                srcw.ap[1] = [-2256, nbl]
                ringB.dma_start(dstw, srcw)

            def tabh(hh, ci, cast_act=False):
                # T1h chunk: matmul -> cast -> DRAM store -> diagonal gather
                s = hs[hh]
                ringA = ringB = nc.sync if hh == 0 else nc.gpsimd
                (q0, qn) = QCHUNKS[ci]
                tp = ps_s.tile([128, 3, 512], F32, tag="s", name=f"tph{hh}_{ci}")
                nc.tensor.matmul(
                    tp[0:95, 0, 0:qn], rhv_sb[64:128, :],
                    s["rq"][64:128, q0 : q0 + qn], start=True, stop=True,
                )
                if cast_act:
                    nc.scalar.activation(
                        s["t1sb"][:, q0 : q0 + qn], tp[0:95, 0, 0:qn], Copy
                    )
                else:
                    nc.vector.tensor_copy(s["t1sb"][:, q0 : q0 + qn], tp[0:95, 0, 0:qn])
                ringA.dma_start(
                    t1d[hh][0:95, q0 : q0 + qn], s["t1sb"][0:95, q0 : q0 + qn]
                )
                (ba, bb) = PIECES[ci]
                nbl = bb - ba
                # rel_h: rq[j, (h,w)] = T1h[47-h+j, 48h+w], h in [ba, bb)
                dsth = s["rq"][0:48, 48 * ba : 48 * bb].rearrange(
                    "p (h w) -> p h w", w=48
                )
                srch = t1d[hh][47 - ba : 95 - ba, 48 * ba : 48 * bb].rearrange(
                    "j (h w) -> j h w", w=48
                )
                srch.ap[1] = [-2256, nbl]
                ringB.dma_start(dsth, srch)

            def relw_exp(hh, half):
                # un-permuting (w-major -> q-major) copy of the already-
                # exponentiated rel_w rows, on the DVE
                s = hs[hh]
                h0 = half * 24  # h-blocks 0:24/24:48 -> cols 0:1152/1152:2304
                co, cn = h0 * 48, 1152
                nc.vector.tensor_copy(
                    s["expw"][:, co : co + cn].rearrange("p (h w) -> p h w", w=48),
                    s["relw"][:, :].rearrange("p (w h) -> p h w", w=48)[:, h0 : h0 + 24, :],
                )

            def pats_copy(hh, co, cn):
                # 3 row-rotations (k-tile offsets 0/32/16) of exp_relw
                s = hs[hh]
                ndma = 0
                for j, off in enumerate((0, 32, 16)):
                    p = 0
                    while p < 128:
                        r0 = (p + off) % 48
                        n = min(48 - r0, 128 - p)
                        eng = (nc.sync, nc.gpsimd)[(hh + ndma) % 2]
                        eng.dma_start(
                            s["pats"][p : p + n, j, co : co + cn],
                            s["expw"][r0 : r0 + n, co : co + cn],
                        )
                        p += n
                        ndma += 1

            # ---- main loop machinery (shared across heads) ----
            # PV pipeline: each chunk's PVs run during the NEXT chunk, one
            # full q-block (all 18 k-tiles, sequential PSUM accumulation)
            # per score group. Interleaving accumulation regions within a
            # PSUM bank corrupts all but the last region, so each block's
            # 18-matmul accumulation must be contiguous in the PE stream.
            st = {"pend": []}

            def epilogue(e):
                hh, ci, q0, qn, o_ps = e["hh"], e["ci"], e["q0"], e["qn"], e["o_ps"]
                nb = qn // 128
                rect = epool.tile([128, 4], F32, tag="rect", name=f"rect{hh}_{ci}")
                ot = epool.tile([128, 256], F32, tag="ot", name=f"ot{hh}_{ci}")
                for b in range(nb):
                    nc.vector.reciprocal(
                        rect[:, b : b + 1],
                        o_ps[:, 128 * b + 64 : 128 * b + 65],
                    )
                    nc.vector.tensor_scalar_mul(
                        ot[:, 64 * b : 64 * (b + 1)],
                        o_ps[:, 128 * b : 128 * b + 64],
                        rect[:, b : b + 1],
                    )
                dmae = nc.sync if ci % 2 == 0 else nc.gpsimd
                r0 = hh * HW + q0
                dmae.dma_start(
                    out_t[r0 : r0 + qn, :].rearrange("(b p) d -> p b d", p=128),
                    ot[:, 0 : 64 * nb].rearrange("p (b d) -> p b d", d=64),
                )

            def pv_step(last_mm):
                if not st["pend"]:
                    return
                e = st["pend"][0]
                b, o_ps, p2s, vt = e["b"], e["o_ps"], e["p2s"], e["vt"]
                for kt in range(KT):
                    g, j = divmod(kt, 3)
                    pv = nc.tensor.matmul(
                        o_ps[:, 128 * b : 128 * b + 65],
                        p2s[g][:, j, 128 * b : 128 * b + 128],
                        vt[:, kt * 65 : (kt + 1) * 65],
                        start=(kt == 0), stop=(kt == KT - 1),
                    )
                    if last_mm is not None:
                        add_dep_helper(pv.ins, last_mm.ins, sync=False,
                                       reason="pv after score mms")
                e["b"] += 1
                if e["b"] == e["qn"] // 128:
                    epilogue(e)
                    st["pend"].pop(0)

            def chunk(hh, ci, inserts=None):
                s = hs[hh]
                (q0, qn) = QCHUNKS[ci]
                o_ps = ps_o.tile([128, 512], F32, tag="o", name=f"o{hh}_{ci}")
                p2s = []
                for g in range(NG):
                    if inserts:
                        inserts.pop(0)()
                    s_ps = ps_s.tile([128, 3, 512], F32, tag="s",
                                     name=f"s{hh}_{ci}_{g}")
                    last_mm = None
                    for j in range(3):
                        kt = 3 * g + j
                        last_mm = nc.tensor.matmul(
                            s_ps[:, j, 0:qn],
                            s["lhsT"][:, kt * 128 : (kt + 1) * 128],
                            s["rq"][:, q0 : q0 + qn],
                            start=True, stop=True,
                        )
                    pv_step(last_mm)
                    if SCH_EVERY and (ci * NG + g) % SCH_EVERY == 2:
                        # Schraudolph exp on DVE (every 5th group)
                        p1i = p1ipool.tile([128, 3, 512], I16, tag="p1i")
                        nc.vector.tensor_scalar(
                            p1i[:, :, 0:qn], s_ps[:, :, 0:qn],
                            SCH_C1, SCH_C2, MULT, ADD,
                        )
                        p1v = p1i.bitcast(BF16)
                    else:
                        p1 = p1pool.tile([128, 3, 512], BF16, tag="p1")
                        nc.scalar.activation(p1[:, :, 0:qn], s_ps[:, :, 0:qn], Exp)
                        p1v = p1
                    p2 = p2pool.tile([128, 3, 512], BF16, tag="p2")
                    nc.vector.tensor_mul(
                        p2[:, :, 0:qn], p1v[:, :, 0:qn],
                        s["pats"][:, :, q0 : q0 + qn],
                    )
                    p2s.append(p2)
                st["pend"].append(dict(hh=hh, ci=ci, q0=q0, qn=qn, o_ps=o_ps,
                                       p2s=p2s, vt=s["vt"], b=0))

            def finish_all():
                while st["pend"]:
                    pv_step(None)

            # ---- schedule ----
            ph1_tiles(0)
            ph1_tiles(1)
            # preload the ACT exp table while DMAs are in flight
            warm = epool.tile([1, 64], BF16, tag="warm")
            nc.scalar.activation(warm[0:1, 0:64], ones1[0:1, 0:64], Exp)
            # startup-critical loads only: first halves of Q layouts + the
            # first 3 k-tiles of the K stack; the rest streams in at slots
            # 0-1 so the rel-table stores aren't queued behind them.
            ph1_load_q_min(0)
            ph1_load_lhs(0, 0, 384)
            # prep chains: T1h pieces 0-1 gate the first score matmul (casts
            # on the idle ACT; Copy shares the Exp table set); the T2w chain
            # (casts on the idle DVE) gates the first p2 multiply, with a
            # full chunk of score-matmul runway before that.
            tabh(0, 0, cast_act=True)
            tabh(0, 1, cast_act=True)
            for c in range(5):
                tabw(0, c)
            relw_exp(0, 0)
            pats_copy(0, 0, 1152)

            nop = lambda: None
            ins0 = [
                # during head-0 chunk 0 (6 slots)
                lambda: (ph1_load_q_rest(0), ph1_load_lhs(0, 384, 1152),
                         ph1_load_vt(0)),
                lambda: (ph1_load_lhs(0, 1152, HW), ph1_load_q(1)),
                lambda: tabh(0, 2),
                lambda: (ph1_load_lhs(1), ph1_load_vt(1)),
                lambda: relw_exp(0, 1),
                lambda: pats_copy(0, 1152, 1152),
                # chunk 1
                lambda: tabw(1, 0),
                nop,
                lambda: tabw(1, 1),
                nop,
                lambda: tabw(1, 2),
                lambda: tabh(0, 3),
                # chunk 2
                lambda: tabw(1, 3),
                nop,
                lambda: tabw(1, 4),
                lambda: tabh(0, 4),
                lambda: relw_exp(1, 0),
                nop,
                # chunk 3
                lambda: pats_copy(1, 0, 1152),
                nop,
                lambda: tabh(1, 0),
                lambda: relw_exp(1, 1),
                nop,
                lambda: tabh(1, 1),
                # chunk 4
                lambda: pats_copy(1, 1152, 1152),
                nop,
                lambda: tabh(1, 2),
                nop,
                lambda: tabh(1, 3),
                nop,
            ]
            ins1 = [
                lambda: tabh(1, 4),
            ] + [nop] * 29

            for ci in range(5):
                chunk(0, ci, ins0)
            for ci in range(5):
                chunk(1, ci, ins1)
            finish_all()

    nc.compile()
    return nc


def _get_nc():
    global _NC
    if _NC is None:
        _NC = _build_nc()
    return _NC


def _host_prep(q, k, v, rel_pos_h, rel_pos_w):
    q2 = np.asarray(q, np.float32).reshape(HW, NH * DH)
    k2 = np.asarray(k, np.float32).reshape(HW, NH * DH)
    v2 = np.asarray(v, np.float32).reshape(HW, NH * DH)
    rph = np.asarray(rel_pos_h, np.float32)
    rpw = np.asarray(rel_pos_w, np.float32)

    ar = np.arange(48)
    # reversed rel tables, x8 cancels the 0.125 q scale
    rhv = np.ascontiguousarray((8.0 * rph[::-1]).T).astype(BF)   # (64, 95)
    rwv = np.ascontiguousarray((8.0 * rpw[::-1]).T).astype(BF)
    kk = np.arange(HW)
    eh = np.zeros((64, HW), np.float32)
    eh[:48] = kk[None, :] // 48 == ar[:, None]
    eh = eh.astype(BF)

    onecol = np.ones((HW, 1), np.float32)
    in_maps = []
    for c in range(N_CORES):
        sl = slice(c * 128, (c + 1) * 128)
        qs = (q2[:, sl].T * 0.125).astype(BF)                    # (128, HW)
        qw = np.ascontiguousarray(
            qs.reshape(128, 48, 48).transpose(0, 2, 1)
        ).reshape(128, HW)                                       # w-major cols
        ks = k2[:, sl].T.astype(BF)
        lhs_p = np.zeros((128, HPC, HW), BF)
        rqs_p = np.zeros((80, HPC, HW), BF)
        rqw_p = np.zeros((64, HPC, HW), BF)
        vparts = []
        for hh in range(HPC):
            r0, r1 = hh * 64, (hh + 1) * 64
            lhs_p[0:64, hh, :] = eh
            lhs_p[64:128, hh, :] = ks[r0:r1]
            rqs_p[16:80, hh, :] = qs[r0:r1]
            rqw_p[:, hh, :] = qw[r0:r1]
            vh = v2[:, c * 128 + hh * 64 : c * 128 + (hh + 1) * 64]
            va = np.concatenate([vh, onecol], axis=1)            # (HW, 65)
            vparts.append(va.reshape(KT, 128, 65).transpose(1, 0, 2).reshape(128, KT * 65))
        v_til = np.concatenate(vparts, axis=1).astype(BF)        # (128, 2*18*65)
        in_maps.append(
            dict(
                lhs_p=lhs_p.reshape(128, HPC * HW),
                rqs_p=rqs_p.reshape(80, HPC * HW),
                rqw_p=rqw_p.reshape(64, HPC * HW),
                v_til=v_til, rhv=rhv, rwv=rwv,
            )
        )
    return in_maps


def _assemble(results):
    # out_t per core: [HPC*HW, 64] f32, head hh in rows hh*HW:(hh+1)*HW
    full = np.empty((HW, NH * DH), np.float32)
    for c, r in enumerate(results):
        o = np.asarray(r["out_t"], np.float32)
        for hh in range(HPC):
            full[:, c * 128 + hh * 64 : c * 128 + (hh + 1) * 64] = \
                o[hh * HW : (hh + 1) * HW, :]
    return full.reshape(1, H, W, NH * DH)


def kernel(q, k, v, rel_pos_h, rel_pos_w):
    nc = _get_nc()
    in_maps = _host_prep(q, k, v, rel_pos_h, rel_pos_w)
    res = run_bass_kernel_spmd(nc, in_maps, core_ids=list(range(N_CORES)))
    return _assemble(res.results)
